# revision 16
# baseline (speedup 1.0000x reference)
"""Trainium2 Bass kernel for a dense transformer block (B=2, T=2048, C=1024,
H=16 heads, HS=64, FF=4096, fp32), SPMD across 8 NeuronCores.

Sharding strategy
-----------------
- LayerNorms + FFN + proj: sequence-parallel. Core c owns 512 tokens
  (rows 512c..512c+511 of the flattened [4096, 1024] activation).
- Attention: head-parallel. Core c owns heads 2c and 2c+1 over all tokens.
- Collectives (all cheap, ~1-2MB/rank):
  1. AllGather of h^T (LN1 output, transposed) so every core can compute
     Q/K/V for its heads over all 4096 tokens.
  2. AllToAll of att^T to re-shard from head-sharded to token-sharded for
     the output projection.
  The final output needs no collective: each core returns its token chunk
  and the host concatenates.

Numerics: all matmuls run as float32r (full PE rate at free-dim>=256,
~16x tighter than bf16); everything else fp32. LayerNorm scale/bias and
the per-head attention scale p^-0.5 are folded into the weight matrices
on the host, so on-device LN is just (x - mean) * rstd.

Layout convention: activations that feed matmul contractions over
channels are kept transposed ([channel, token]) so no operand ever needs
an on-the-fly transpose except LN outputs (32 PE transposes each).
"""

import os
import numpy as np

B, T, C = 2, 2048, 1024
H, HS = 16, 64
FF = 4 * C
EPS = 1e-5
NCORE = 8
TOK = B * T            # 4096 flattened tokens
CHUNK = TOK // NCORE   # 512 tokens per core
P = 128
NTT = CHUNK // P       # 4 token tiles of 128 per core
NG = C // P            # 8 channel chunks
NF = FF // P           # 32 ff slices
LH = 2                 # local heads per core

_BUILT = None


def _build():
    import concourse.bass as bass
    import concourse.tile as tile
    from concourse import bacc, mybir
    from concourse.masks import make_identity
    from contextlib import ExitStack

    f32 = mybir.dt.float32
    f32r = mybir.dt.float32r
    Alu = mybir.AluOpType
    Act = mybir.ActivationFunctionType

    nc = bacc.Bacc("TRN2", target_bir_lowering=False, debug=False,
                   num_devices=NCORE)

    xc = nc.dram_tensor("xc", [CHUNK, C], f32, kind="ExternalInput").ap()
    wqkv = nc.dram_tensor("wqkv", [C, 3 * P], f32r, kind="ExternalInput").ap()
    bqk = nc.dram_tensor("bqk", [P, LH], f32, kind="ExternalInput").ap()
    bv = nc.dram_tensor("bv", [P, 1], f32, kind="ExternalInput").ap()
    wproj = nc.dram_tensor("wproj", [C, C], f32r, kind="ExternalInput").ap()
    w1 = nc.dram_tensor("w1", [C, FF], f32r, kind="ExternalInput").ap()
    bff1 = nc.dram_tensor("bff1", [P, NF], f32, kind="ExternalInput").ap()
    w2 = nc.dram_tensor("w2", [FF, C], f32r, kind="ExternalInput").ap()
    out = nc.dram_tensor("out", [CHUNK, C], f32, kind="ExternalOutput").ap()

    # collective bounce buffers (internal DRAM; outputs must be Shared)
    h_bounce = nc.dram_tensor("h_bounce", [C, CHUNK], f32r)
    hT_all = nc.dram_tensor("hT_all", [NCORE * C, CHUNK], f32r,
                            addr_space="Shared")
    attT_bounce = nc.dram_tensor("attT_bounce", [NCORE, P, CHUNK], f32r)
    attT_recv = nc.dram_tensor("attT_recv", [NCORE, P, CHUNK], f32r)
    groups = [list(range(NCORE))]

    with tile.TileContext(nc) as tc, ExitStack() as top:
        const = top.enter_context(tc.tile_pool(name="const", bufs=1))
        persist = top.enter_context(tc.tile_pool(name="persist", bufs=1))
        ps = top.enter_context(tc.tile_pool(name="ps", bufs=8, space="PSUM"))

        ident = const.tile([P, P], f32)
        make_identity(nc, ident)
        ones1_f = const.tile([1, 64], f32)
        nc.vector.memset(ones1_f, 1.0)
        ones1 = const.tile([1, 64], f32r)
        nc.vector.tensor_copy(ones1, ones1_f)
        onescol_f = const.tile([P, TOK // P], f32)
        nc.vector.memset(onescol_f, 1.0)
        eps_sb = const.tile([P, 1], f32)
        nc.vector.memset(eps_sb, EPS)

        xc_sb = persist.tile([P, NTT, C], f32)
        xmid_sb = persist.tile([P, NTT, C], f32)
        wqkv_sb = persist.tile([P, NG, 3 * P], f32r)
        bqk_sb = persist.tile([P, LH], f32)
        bv_sb = persist.tile([P, 1], f32)
        bff1_sb = persist.tile([P, NF], f32)

        nc.sync.dma_start(out=wqkv_sb,
                          in_=wqkv.rearrange("(g p) m -> p g m", p=P))
        nc.sync.dma_start(out=bqk_sb, in_=bqk)
        nc.sync.dma_start(out=bv_sb, in_=bv)
        nc.sync.dma_start(out=bff1_sb, in_=bff1)

        def layernorm_tile(pool, src_ap):
            """src_ap: [P, C] fp32 in SBUF -> returns normalized [P, C] tile."""
            stats = pool.tile([P, 2, 6], f32, tag="ln_stats")
            nc.vector.bn_stats(out=stats[:, 0, :], in_=src_ap[:, 0:512])
            nc.vector.bn_stats(out=stats[:, 1, :], in_=src_ap[:, 512:1024])
            mv = pool.tile([P, 2], f32, tag="ln_mv")
            nc.vector.bn_aggr(out=mv, in_=stats)
            rstd = pool.tile([P, 1], f32, tag="ln_rstd")
            nc.scalar.activation(rstd, mv[:, 1:2], Act.Sqrt, bias=eps_sb)
            nc.vector.reciprocal(rstd, rstd)
            hn = pool.tile([P, C], f32, tag="ln_out")
            nc.vector.tensor_scalar(hn, src_ap, mv[:, 0:1], rstd,
                                    Alu.subtract, Alu.mult)
            return hn

        # ---------------- Stage A: LN1 + transpose + AllGather --------------
        with ExitStack() as sa:
            lnp = sa.enter_context(tc.tile_pool(name="lnp", bufs=3))
            for jt in range(NTT):
                nc.sync.dma_start(out=xc_sb[:, jt, :],
                                  in_=xc[P * jt:P * (jt + 1), :])
                hn = layernorm_tile(lnp, xc_sb[:, jt, :])
                for g in range(NG):
                    tp = ps.tile([P, P], f32, tag="bank")
                    nc.tensor.transpose(tp, hn[:, P * g:P * (g + 1)], ident)
                    hb = lnp.tile([P, P], f32r, tag="htout")
                    nc.vector.tensor_copy(hb, tp)
                    nc.sync.dma_start(
                        out=h_bounce[P * g:P * (g + 1), P * jt:P * (jt + 1)],
                        in_=hb)
            nc.gpsimd.collective_compute(
                "AllGather", Alu.bypass, replica_groups=groups,
                ins=[h_bounce[:, :]], outs=[hT_all[:, :]])

        # ---------------- Stage B: QKV + attention --------------------------
        with ExitStack() as sb:
            qkp = sb.enter_context(tc.tile_pool(name="qkp", bufs=1))
            htp = sb.enter_context(tc.tile_pool(name="htp", bufs=10))
            vtp = sb.enter_context(tc.tile_pool(name="vtp", bufs=2))

            qkT = qkp.tile([P, LH, TOK], f32r)   # rows 0:64 Q^T, 64:128 K^T
            # K^T copied down to base partition 0 (matmul needs equal
            # base_partition on both operands; K bias is dropped — softmax
            # is invariant to per-query score offsets)
            kT = qkp.tile([64, LH, TOK], f32r)
            Vsb = qkp.tile([P, TOK // P, 132], f32r)
            onescol3 = onescol_f.rearrange("p (a b) -> p a b", b=1)
            nc.vector.tensor_copy(Vsb[:, :, 64:65], onescol3)
            nc.vector.tensor_copy(Vsb[:, :, 130:131], onescol3)

            for rr in range(NCORE):
                hts = []
                for g in range(NG):
                    ht = htp.tile([P, CHUNK], f32r, tag="ht")
                    nc.sync.dma_start(
                        out=ht, in_=hT_all[C * rr + P * g:C * rr + P * (g + 1), :])
                    hts.append(ht)
                for hp in range(LH):
                    psA = ps.tile([P, CHUNK], f32, tag="bank")
                    for g in range(NG):
                        nc.tensor.matmul(
                            psA, wqkv_sb[:, g, P * hp:P * (hp + 1)],
                            hts[g], start=(g == 0), stop=(g == NG - 1))
                    nc.vector.tensor_scalar_add(
                        qkT[:, hp, CHUNK * rr:CHUNK * (rr + 1)], psA,
                        bqk_sb[:, hp:hp + 1])
                    nc.sync.dma_start(
                        out=kT[:, hp, CHUNK * rr:CHUNK * (rr + 1)],
                        in_=qkT[64:128, hp, CHUNK * rr:CHUNK * (rr + 1)])
                psV = ps.tile([P, CHUNK], f32, tag="bank")
                for g in range(NG):
                    nc.tensor.matmul(psV, wqkv_sb[:, g, 2 * P:3 * P],
                                     hts[g], start=(g == 0),
                                     stop=(g == NG - 1))
                vt = vtp.tile([P, CHUNK], f32, tag="vt")
                nc.vector.tensor_scalar_add(vt, psV, bv_sb)
                for tt in range(NTT):
                    tpv = ps.tile([P, P], f32, tag="bank")
                    nc.tensor.transpose(tpv, vt[:, P * tt:P * (tt + 1)], ident)
                    vdst = Vsb[:, NTT * rr + tt, :].rearrange(
                        "p (a b) -> p a b", a=2)[:, :, 0:64]
                    vsrc = tpv.rearrange("p (a b) -> p a b", a=2)
                    nc.vector.tensor_copy(vdst, vsrc)

            # attention: per local head hp, batch b, query tile jq (512 wide)
            atp = sb.enter_context(tc.tile_pool(name="atp", bufs=4))
            for hp in range(LH):
                for b in range(B):
                    base_t = T * b
                    for jq in range(4):
                        q0 = base_t + 512 * jq
                        nk = 4 * (jq + 1)
                        psPV = ps.tile([65, 512], f32, tag="bank")
                        for ik in range(nk):
                            k0 = base_t + P * ik
                            psS = ps.tile([P, 512], f32, tag="bank")
                            nc.tensor.matmul(
                                psS, kT[:, hp, k0:k0 + P],
                                qkT[0:64, hp, q0:q0 + 512],
                                start=True, stop=True)
                            pt = atp.tile([P, 512], f32r, tag="pt")
                            nc.scalar.activation(pt, psS, Act.Exp)
                            diag = 512 * jq - P * ik
                            if diag < P:  # diagonal block: causal mask
                                nc.gpsimd.affine_select(
                                    out=pt, in_=pt, pattern=[[1, 512]],
                                    compare_op=Alu.is_ge, fill=0.0,
                                    base=diag, channel_multiplier=-1)
                            nc.tensor.matmul(
                                psPV, Vsb[:, (base_t // P) + ik,
                                            66 * hp:66 * hp + 65],
                                pt, start=(ik == 0), stop=(ik == nk - 1))
                        rec = atp.tile([1, 512], f32r, tag="rec")
                        with nc.allow_low_precision(
                                reason="f32r reciprocal feeds f32r matmul"):
                            nc.vector.reciprocal(rec, psPV[64:65, :])
                        psBC = ps.tile([64, 512], f32, tag="bank")
                        nc.tensor.matmul(psBC, ones1, rec,
                                         start=True, stop=True)
                        bc = atp.tile([64, 512], f32, tag="bc")
                        nc.scalar.copy(bc, psBC)
                        att = atp.tile([64, 512], f32r, tag="attout")
                        nc.vector.tensor_mul(att, psPV[0:64, :], bc)
                        nc.sync.dma_start(
                            out=attT_bounce[4 * b + jq,
                                            64 * hp:64 * (hp + 1), :],
                            in_=att)
            nc.gpsimd.collective_compute(
                "AllToAll", Alu.bypass, replica_groups=groups,
                ins=[attT_bounce[:, :, :]], outs=[attT_recv[:, :, :]])

        # ---------------- Stage C: proj + residual --------------------------
        with ExitStack() as sc:
            prp = sc.enter_context(tc.tile_pool(name="prp", bufs=6))
            for n in range(2):
                psj = [ps.tile([P, 512], f32, tag="bank", name=f"psj{n}_{jt}")
                       for jt in range(NTT)]
                for g in range(NG):
                    wp = prp.tile([P, 512], f32r, tag="wp")
                    nc.sync.dma_start(
                        out=wp, in_=wproj[P * g:P * (g + 1),
                                          512 * n:512 * (n + 1)])
                    for jt in range(NTT):
                        at = prp.tile([P, P], f32r, tag="at")
                        nc.sync.dma_start(
                            out=at, in_=attT_recv[g, :, P * jt:P * (jt + 1)])
                        nc.tensor.matmul(psj[jt], at, wp,
                                         start=(g == 0), stop=(g == NG - 1))
                for jt in range(NTT):
                    nc.vector.tensor_add(
                        xmid_sb[:, jt, 512 * n:512 * (n + 1)], psj[jt],
                        xc_sb[:, jt, 512 * n:512 * (n + 1)])

        # ---------------- Stage D: LN2 + FFN + residual ---------------------
        with ExitStack() as sd:
            ffp = sd.enter_context(tc.tile_pool(name="ffp", bufs=1))
            lnp2 = sd.enter_context(tc.tile_pool(name="lnp2", bufs=3))
            w1p = sd.enter_context(tc.tile_pool(name="w1p", bufs=6))
            w2p = sd.enter_context(tc.tile_pool(name="w2p", bufs=3))
            outp = sd.enter_context(tc.tile_pool(name="outp", bufs=3))

            h2T = ffp.tile([P, NG, CHUNK], f32r)
            ff1T = ffp.tile([P, NF, CHUNK], f32r)

            for jt in range(NTT):
                hn2 = layernorm_tile(lnp2, xmid_sb[:, jt, :])
                for g in range(NG):
                    tp = ps.tile([P, P], f32, tag="bank")
                    nc.tensor.transpose(tp, hn2[:, P * g:P * (g + 1)], ident)
                    nc.vector.tensor_copy(
                        h2T[:, g, P * jt:P * (jt + 1)], tp)

            for f in range(NF):
                psF = ps.tile([P, CHUNK], f32, tag="bank")
                for g in range(NG):
                    w1t = w1p.tile([P, P], f32r, tag="w1t")
                    nc.sync.dma_start(
                        out=w1t, in_=w1[P * g:P * (g + 1), P * f:P * (f + 1)])
                    nc.tensor.matmul(psF, w1t, h2T[:, g, :],
                                     start=(g == 0), stop=(g == NG - 1))
                nc.scalar.activation(ff1T[:, f, :], psF, Act.Relu,
                                     bias=bff1_sb[:, f:f + 1])

            for n in range(2):
                psj = [ps.tile([P, 512], f32, tag="bank", name=f"psj{n}_{jt}")
                       for jt in range(NTT)]
                for q in range(NF):
                    w2t = w2p.tile([P, 512], f32r, tag="w2t")
                    nc.sync.dma_start(
                        out=w2t, in_=w2[P * q:P * (q + 1),
                                        512 * n:512 * (n + 1)])
                    for jt in range(NTT):
                        nc.tensor.matmul(
                            psj[jt], ff1T[:, q, P * jt:P * (jt + 1)],
                            w2t, start=(q == 0), stop=(q == NF - 1))
                for jt in range(NTT):
                    ot = outp.tile([P, 512], f32, tag="outt")
                    nc.vector.tensor_add(ot, psj[jt],
                                         xmid_sb[:, jt, 512 * n:512 * (n + 1)])
                    nc.sync.dma_start(
                        out=out[P * jt:P * (jt + 1), 512 * n:512 * (n + 1)],
                        in_=ot)

    nc.compile()
    return nc


def _prepare_inputs(x, Wq, Wk, Wv, p, Wproj, W1, W2,
                    ln1_w, ln1_b, ln2_w, ln2_b):
    f = np.float32
    x = np.asarray(x, f).reshape(TOK, C)
    Wq, Wk, Wv = (np.asarray(a, f) for a in (Wq, Wk, Wv))
    p = np.asarray(p, f)
    Wproj = np.asarray(Wproj, f)
    W1, W2 = np.asarray(W1, f), np.asarray(W2, f)
    ln1_w, ln1_b = np.asarray(ln1_w, f), np.asarray(ln1_b, f)
    ln2_w, ln2_b = np.asarray(ln2_w, f), np.asarray(ln2_b, f)

    s = p.astype(np.float64) ** -0.5
    s = s.astype(f)

    w1_f = (ln2_w[:, None] * W1).astype(f)
    bff1 = ln2_b @ W1
    bff1 = np.ascontiguousarray(bff1.reshape(NF, P).T.astype(f))

    in_maps = []
    for c in range(NCORE):
        h0, h1 = 2 * c, 2 * c + 1
        blocks = []
        bqk_cols = []
        for h in (h0, h1):
            wq_f = ln1_w[:, None] * Wq[h] * s[h]
            wk_f = ln1_w[:, None] * Wk[h]
            blocks.append(np.concatenate([wq_f, wk_f], axis=1))
            # K bias intentionally zero: softmax is invariant to it
            bqk_cols.append(np.concatenate(
                [s[h] * (ln1_b @ Wq[h]), np.zeros(HS, f)]))
        wv_f = np.concatenate(
            [ln1_w[:, None] * Wv[h0], ln1_w[:, None] * Wv[h1]], axis=1)
        blocks.append(wv_f)
        wqkv_c = np.ascontiguousarray(
            np.concatenate(blocks, axis=1).astype(f))
        bqk_c = np.ascontiguousarray(np.stack(bqk_cols, axis=1).astype(f))
        bv_c = np.ascontiguousarray(np.concatenate(
            [ln1_b @ Wv[h0], ln1_b @ Wv[h1]])[:, None].astype(f))
        in_maps.append({
            "xc": np.ascontiguousarray(x[CHUNK * c:CHUNK * (c + 1)]),
            "wqkv": wqkv_c,
            "bqk": bqk_c,
            "bv": bv_c,
            "wproj": Wproj,
            "w1": w1_f,
            "bff1": bff1,
            "w2": W2,
        })
    return in_maps


def kernel(**inputs):
    global _BUILT
    from concourse.bass_utils import run_bass_kernel_spmd

    if _BUILT is None:
        _BUILT = _build()
    in_maps = _prepare_inputs(**inputs)
    trace = bool(int(os.environ.get("BASSK_TRACE", "0")))
    res = run_bass_kernel_spmd(_BUILT, in_maps, list(range(NCORE)),
                               trace=trace)
    if trace:
        kernel.last_exec_time_ns = res.exec_time_ns
    out = np.concatenate([res.results[c]["out"] for c in range(NCORE)], axis=0)
    return out.reshape(B, T, C).astype(np.float32)
